# revision 6
# baseline (speedup 1.0000x reference)
"""Cross-attention Trainium2 kernel, sharded over heads across 8 NeuronCores.

Core i computes head i for both batches:
  q/k/v projections (f32r matmuls), scores^T = k^T q scaled+masked,
  exp on ScalarE, attention accumulated over k-chunks with a ones-column
  on V^T supplying the softmax denominator, normalization, and the
  partial output projection Wout[:, head]^T @ attn.
Host sums the 8 partial outputs and adds bout.
"""

import sys

sys.path.insert(0, "/opt/trn_rl_repo")

import numpy as np

import concourse.bacc as bacc
import concourse.tile as tile
from concourse import masks, mybir
from concourse.bass_utils import run_bass_kernel_spmd

HEADS = 8
DH = 160
C = 1280
B = 2
S = 4096
NQ = 512  # q-block size
NP = 256  # projection s-block size
SCALE = DH ** -0.5

_cache = {}


def _build(s=S, reps=1):
    key = (s, reps)
    if key in _cache:
        return _cache[key]
    f32, f32r = mybir.dt.float32, mybir.dt.float32r
    KC = s // 128  # k chunks
    QBN = s // NQ  # q blocks
    SBN = s // NP  # projection s-blocks
    CT = C // 128  # contraction tiles for projections

    nc = bacc.Bacc("TRN2", target_bir_lowering=False, debug=False, num_devices=HEADS)
    d_x = nc.dram_tensor("x", [B, C, s], f32r, kind="ExternalInput").ap()
    d_c = nc.dram_tensor("c", [B, C, s], f32r, kind="ExternalInput").ap()
    d_msk = nc.dram_tensor("msk", [B, s], f32, kind="ExternalInput").ap()
    d_wqt = nc.dram_tensor("wqt", [C, DH], f32r, kind="ExternalInput").ap()
    d_wkt = nc.dram_tensor("wkt", [C, 128], f32r, kind="ExternalInput").ap()
    d_wvt = nc.dram_tensor("wvt", [C, 128], f32r, kind="ExternalInput").ap()
    d_w2t = nc.dram_tensor("w2t", [C, 64], f32r, kind="ExternalInput").ap()
    d_wot = nc.dram_tensor("wot", [DH, C], f32r, kind="ExternalInput").ap()
    d_out = nc.dram_tensor("out", [B, C, s], f32, kind="ExternalOutput").ap()

    x_r = d_x.rearrange("b (t p) s -> b p t s", p=128)
    c_r = d_c.rearrange("b (t p) s -> b p t s", p=128)

    with tile.TileContext(nc) as tc:
        with (
            tc.tile_pool(name="wp", bufs=1) as wp,
            tc.tile_pool(name="big", bufs=1) as big,
            tc.tile_pool(name="stream", bufs=2) as stream,
            tc.tile_pool(name="expp", bufs=6) as expp,
            tc.tile_pool(name="smal", bufs=2) as smal,
            tc.tile_pool(name="outp", bufs=3) as outp,
            tc.tile_pool(name="psS", bufs=3, space="PSUM") as psS,   # 3 banks
            tc.tile_pool(name="psa", bufs=1, space="PSUM") as psa,   # 2 banks
            tc.tile_pool(name="pso", bufs=2, space="PSUM") as pso,   # 2 banks
        ):
            # ---- constants / weights ----
            wqt = wp.tile([128, CT, DH], f32r)
            nc.sync.dma_start(out=wqt, in_=d_wqt.rearrange("(t p) d -> p t d", p=128))
            wkt = wp.tile([128, CT, 128], f32r)
            nc.sync.dma_start(out=wkt, in_=d_wkt.rearrange("(t p) d -> p t d", p=128))
            wvt = wp.tile([128, CT, 128], f32r)
            nc.sync.dma_start(out=wvt, in_=d_wvt.rearrange("(t p) d -> p t d", p=128))
            w2t = wp.tile([128, CT, 64], f32r)
            nc.sync.dma_start(out=w2t, in_=d_w2t.rearrange("(t p) d -> p t d", p=128))
            woA = wp.tile([128, C], f32r)
            nc.sync.dma_start(out=woA, in_=d_wot[0:128, :])
            woB = wp.tile([32, C], f32r)
            nc.sync.dma_start(out=woB, in_=d_wot[128:160, :])
            msk = wp.tile([128, B, KC], f32)
            nc.sync.dma_start(out=msk, in_=d_msk.rearrange("b (t p) -> p b t", p=128))
            ones_col = wp.tile([1, 128], f32)
            nc.vector.memset(ones_col, 1.0)
            ident = wp.tile([128, 128], f32)
            masks.make_identity(nc, ident[:])

            for _ in range(reps):
                for b in range(B):
                    kA = big.tile([128, s], f32r, tag="kA")
                    qA = big.tile([128, s], f32r, tag="qA")
                    kB = big.tile([32, s], f32r, tag="kB")
                    qB = big.tile([32, s], f32r, tag="qB")
                    vT = big.tile([128, KC, DH + 1], f32r, tag="vT")

                    # ---- phase 1: projections, streamed over s ----
                    for si in range(SBN):
                        sl = slice(NP * si, NP * si + NP)
                        ct = stream.tile([128, CT, NP], f32r, tag="ct")
                        nc.sync.dma_start(out=ct, in_=c_r[b, :, :, sl])
                        ht = stream.tile([128, CT, NP], f32r, tag="ht")
                        nc.sync.dma_start(out=ht, in_=x_r[b, :, :, sl])

                        pk = psS.tile([128, NP], f32, tag="ps")
                        for t in range(CT):
                            nc.tensor.matmul(out=pk, lhsT=wkt[:, t, :], rhs=ct[:, t, :],
                                             start=(t == 0), stop=(t == CT - 1))
                        nc.vector.tensor_copy(out=kA[:, sl], in_=pk)

                        p2 = psS.tile([64, NP], f32, tag="ps")
                        for t in range(CT):
                            nc.tensor.matmul(out=p2, lhsT=w2t[:, t, :], rhs=ct[:, t, :],
                                             start=(t == 0), stop=(t == CT - 1))
                        st2 = stream.tile([64, NP], f32, tag="st2")
                        nc.vector.tensor_copy(out=st2, in_=p2)
                        nc.sync.dma_start(out=kB[:, sl],
                                          in_=st2[32:64, :].bitcast(f32r))
                        for j in range(NP // 128):
                            pt2 = psS.tile([128, 32], f32, tag="ps")
                            nc.tensor.transpose(out=pt2,
                                                in_=st2[0:32, 128 * j:128 * j + 128],
                                                identity=ident[0:32, 0:32])
                            kci = (NP * si) // 128 + j
                            nc.vector.tensor_copy(out=vT[:, kci, 128:160], in_=pt2)

                        pv = psS.tile([128, NP], f32, tag="ps")
                        for t in range(CT):
                            nc.tensor.matmul(out=pv, lhsT=wvt[:, t, :], rhs=ct[:, t, :],
                                             start=(t == 0), stop=(t == CT - 1))
                        vst = stream.tile([128, NP], f32, tag="vst")
                        nc.scalar.copy(out=vst, in_=pv)
                        for j in range(NP // 128):
                            pt1 = psS.tile([128, 128], f32, tag="ps")
                            nc.tensor.transpose(out=pt1,
                                                in_=vst[:, 128 * j:128 * j + 128],
                                                identity=ident[:])
                            kci = (NP * si) // 128 + j
                            nc.vector.tensor_copy(out=vT[:, kci, 0:128], in_=pt1)

                        pq = psS.tile([128, NP], f32, tag="ps")
                        for t in range(CT):
                            nc.tensor.matmul(out=pq, lhsT=wqt[:, t, 0:128], rhs=ht[:, t, :],
                                             start=(t == 0), stop=(t == CT - 1))
                        nc.scalar.copy(out=qA[:, sl], in_=pq)

                        pq2 = psS.tile([32, NP], f32, tag="ps")
                        for t in range(CT):
                            nc.tensor.matmul(out=pq2, lhsT=wqt[:, t, 128:160], rhs=ht[:, t, :],
                                             start=(t == 0), stop=(t == CT - 1))
                        nc.scalar.copy(out=qB[:, sl], in_=pq2)

                    nc.vector.memset(vT[:, :, 160:161].bitcast(f32), 1.0)

                    # ---- phase 3: attention + output projection per q-block ----
                    for qb in range(QBN):
                        qsl = slice(NQ * qb, NQ * qb + NQ)
                        pa1 = psa.tile([128, NQ], f32, tag="pa1")
                        pa2 = psa.tile([33, NQ], f32, tag="pa2")
                        for kc in range(KC):
                            ksl = slice(128 * kc, 128 * kc + 128)
                            ps_t = psS.tile([128, NQ], f32, tag="ps")
                            nc.tensor.matmul(out=ps_t, lhsT=kA[:, ksl], rhs=qA[:, qsl],
                                             start=True, stop=False)
                            nc.tensor.matmul(out=ps_t, lhsT=kB[:, ksl], rhs=qB[:, qsl],
                                             start=False, stop=True)
                            et = expp.tile([128, NQ], f32r, tag="et")
                            nc.scalar.activation(out=et, in_=ps_t,
                                                 func=mybir.ActivationFunctionType.Exp,
                                                 bias=msk[:, b, kc:kc + 1], scale=SCALE)
                            nc.tensor.matmul(out=pa1, lhsT=vT[:, kc, 0:128], rhs=et,
                                             start=(kc == 0), stop=(kc == KC - 1))
                            nc.tensor.matmul(out=pa2, lhsT=vT[:, kc, 128:161], rhs=et,
                                             start=(kc == 0), stop=(kc == KC - 1))

                        # normalize: reciprocal of denominator row, broadcast via PE
                        rec = smal.tile([1, NQ], f32, tag="rec")
                        nc.vector.reciprocal(out=rec, in_=pa2[32:33, :])
                        pb = psS.tile([128, NQ], f32, tag="ps")
                        nc.tensor.matmul(out=pb, lhsT=ones_col[:], rhs=rec[:],
                                         start=True, stop=True)
                        bc = smal.tile([128, NQ], f32, tag="bc")
                        nc.scalar.copy(out=bc, in_=pb)
                        a1 = smal.tile([128, NQ], f32r, tag="a1")
                        nc.vector.tensor_mul(a1[:], pa1[:], bc[:])
                        a2 = smal.tile([32, NQ], f32r, tag="a2")
                        nc.vector.tensor_mul(a2[:], pa2[0:32, :], bc[0:32, :])

                        # output projection
                        for oc in range(CT):
                            osl = slice(128 * oc, 128 * oc + 128)
                            po = pso.tile([128, NQ], f32, tag="po")
                            nc.tensor.matmul(out=po, lhsT=woA[:, osl], rhs=a1[:],
                                             start=True, stop=False)
                            nc.tensor.matmul(out=po, lhsT=woB[:, osl], rhs=a2[:],
                                             start=False, stop=True)
                            ot = outp.tile([128, NQ], f32, tag="ot")
                            nc.vector.tensor_copy(out=ot, in_=po)
                            nc.sync.dma_start(out=d_out[b, osl, qsl], in_=ot)

    nc.compile()
    _cache[key] = nc
    return nc


def _prep_inputs(hidden_states, context, mask, Wq, Wk, Wv, Wout):
    x = np.ascontiguousarray(
        np.asarray(hidden_states, dtype=np.float32)[:, :, 0, :])
    c = np.ascontiguousarray(np.asarray(context, dtype=np.float32)[:, :, 0, :])
    msk = np.ascontiguousarray(np.asarray(mask, dtype=np.float32)[:, :, 0, 0])
    Wq = np.asarray(Wq, dtype=np.float32)
    Wk = np.asarray(Wk, dtype=np.float32)
    Wv = np.asarray(Wv, dtype=np.float32)
    Wout = np.asarray(Wout, dtype=np.float32)
    ins = []
    for h in range(HEADS):
        rows = slice(DH * h, DH * h + DH)
        wq, wk, wv = Wq[rows], Wk[rows], Wv[rows]
        ins.append({
            "x": x,
            "c": c,
            "msk": msk,
            "wqt": np.ascontiguousarray(wq.T),
            "wkt": np.ascontiguousarray(wk[0:128].T),
            "wvt": np.ascontiguousarray(wv[0:128].T),
            "w2t": np.ascontiguousarray(
                np.concatenate([wv[128:160], wk[128:160]], axis=0).T),
            "wot": np.ascontiguousarray(Wout[:, rows].T),
        })
    return ins


def kernel(hidden_states, context, mask, Wq, Wk, Wv, Wout, bout):
    nc = _build()
    ins = _prep_inputs(hidden_states, context, mask, Wq, Wk, Wv, Wout)
    res = run_bass_kernel_spmd(nc, ins, core_ids=list(range(HEADS)))
    total = res.results[0]["out"].astype(np.float32)
    for h in range(1, HEADS):
        total = total + res.results[h]["out"]
    total = total + np.asarray(bout, dtype=np.float32)[None, :, None]
    return total[:, :, None, :].astype(np.float32)
